# revision 16
# baseline (speedup 1.0000x reference)
"""Multi-head cross attention (B=32, Nq=16384, Nk=31, d_model=64, H=4) on 8 trn2 cores.

Data parallel over batch (4 per core). Per batch the attention is restructured so
the only large tensor (Q) is streamed once, host-pretransposed to qT [64, Nq] bf16:

  st  = Kblk_aug^T @ qt          Kblk_aug [64,125]: block-diag (K_h/sqrt(dh))^T,
                                 col 124 = 0 (so exp gives a constant ones row)
  et  = exp(st)                  no bias: the 0/1 mask is folded multiplicatively
                                 into onesb/VW below
  r4  = onesb_j^T @ et_j         16 iterations accumulate into one [81,512] PSUM
                                 bank via column-shifted selectors; rows 5j+h =
                                 masked head sums, row 80 = 1 (bias normalizer)
  rinv = 1/r4                    one reciprocal_approx_fast per 16 iterations
  rx  = broadcast(rinv)          SBUF->HBM bounce then per-head broadcast DMAs
                                 build rx [125, 16, 512] (row 124 = 1)
  en  = et * rx                  all-bf16 SBUF multiply (DVE 2x mode)
  u   = VW_aug^T @ en            VW_aug [125,64]: masked V_h @ W_out_h^T blocks,
                                 row 124 = b_out -> u = out + bias
  out copies pack 2 iters into one [128,512] PSUM bank -> one copy per pair.

The loop is software-pipelined over quads (4 iterations): st/exp lead, r4 one
quad behind, mul/u L quads behind, so each matmul's inputs are long since ready
and same-weight matmuls run in adjacent bursts (LDWEIGHTS switch costs ~120ns).
"""

import os
import sys

for _p in ("/opt/trn_rl_repo", "/opt/pypackages",
           "/root/.axon_site/_ro/trn_rl_repo", "/root/.axon_site/_ro/pypackages"):
    if os.path.isdir(_p) and _p not in sys.path:
        sys.path.insert(0, _p)

import math
import numpy as np

import concourse.bass as bass
import concourse.tile as tile
from concourse import bacc, mybir
from concourse.bass_utils import run_bass_kernel_spmd

B, NQ, NK, D = 32, 16384, 31, 64
H, DH = 4, 16
SCALE = math.sqrt(DH)
NCORES = 8
BL = B // NCORES          # batches per core
TQ = 512                  # queries per iteration
NT = NQ // TQ             # iterations per batch (32)
GRP = 16                  # iterations per reciprocal group
NG = NT // GRP            # groups per batch (2)
KB = H * NK               # 124 stacked key rows
KBA = KB + 1              # +1 ones/bias row
NH5 = H + 1               # 4 heads + bias pseudo-head
NPK = GRP * NH5           # 80 packed r4 rows (16 iters x 5)
OSEL = 2 * GRP * NH5      # 160 columns of the padded selector

QCH = 4096                # q columns per input DMA (8 iters)
OCH = 2048                # q columns per output DMA tile (8 iters, row-packed)
LAG = 8                   # quads of pipeline lag between st/exp and mul/u

_PROG_CACHE: dict = {}


def _build_v3():
    f32 = mybir.dt.float32
    bf16 = mybir.dt.bfloat16

    nc = bacc.Bacc("TRN2", target_bir_lowering=False, debug=False, num_devices=NCORES)
    qT = nc.dram_tensor("qT", [BL, D, NQ], bf16, kind="ExternalInput").ap()
    kblk = nc.dram_tensor("kblk", [BL, D, KBA], bf16, kind="ExternalInput").ap()
    vw = nc.dram_tensor("vw", [BL, KBA, D], bf16, kind="ExternalInput").ap()
    onesb = nc.dram_tensor("onesb", [BL, KBA, OSEL], bf16,
                           kind="ExternalInput").ap()
    n_otile = NQ * D // (128 * OCH)     # 4 output tiles per batch
    opk = nc.dram_tensor("opk", [BL, n_otile, 128, OCH], bf16,
                         kind="ExternalOutput").ap()
    ngroups = BL * NG
    # bounce buffer in head-major layout: rb[gi, h] is a contiguous 16KB run,
    # so each broadcast-read descriptor is one fat 16KB line per partition
    rb = nc.dram_tensor("rb", [ngroups, NH5, GRP, TQ], bf16, kind="Internal").ap()
    ones8k = nc.dram_tensor("ones8k", [GRP * TQ], bf16, kind="ExternalInput").ap()

    nquads = BL * NG * (GRP // 4)       # 32 quad steps

    with tile.TileContext(nc) as tc:
        with (
            tc.tile_pool(name="singles", bufs=1) as singles,
            tc.tile_pool(name="qin", bufs=3) as qin_pool,
            tc.tile_pool(name="stp", bufs=2, space="PSUM") as stp_pool,
            tc.tile_pool(name="etp", bufs=2 * LAG + 6) as et_pool,
            tc.tile_pool(name="r4", bufs=2, space="PSUM") as r4_pool,
            tc.tile_pool(name="rinvf", bufs=2) as rinvf_pool,
            tc.tile_pool(name="rinvb", bufs=2) as rinvb_pool,
            tc.tile_pool(name="rxb", bufs=2) as rx_pool,
            tc.tile_pool(name="enp", bufs=4) as en_pool,
            tc.tile_pool(name="u", bufs=2, space="PSUM") as u_pool,
            tc.tile_pool(name="osb", bufs=3) as o_pool,
        ):
            kblk_sb = singles.tile([D, BL, KBA], bf16)
            vw_sb = singles.tile([KBA, BL, D], bf16)
            onesb_sb = singles.tile([KBA, BL, OSEL], bf16)
            for b in range(BL):
                nc.sync.dma_start(out=kblk_sb[:, b, :], in_=kblk[b])
                nc.sync.dma_start(out=vw_sb[:, b, :], in_=vw[b])
                nc.sync.dma_start(out=onesb_sb[:, b, :], in_=onesb[b])

            qin_tiles = {}
            stp_tiles = {}
            etp_tiles = {}
            r4_tiles = {}
            rxb_tiles = {}
            enp_tiles = {}
            osb_tiles = {}
            copy_flip = [0]

            def quad_info(t):
                gi = t // 4                # global group index
                b = gi // NG               # batch on this core
                j0 = (t % 4) * 4           # first group-local iteration
                it0 = (gi % NG) * GRP + j0  # first batch-local iteration
                return gi, b, j0, it0

            def stage_a(t):
                """st matmuls (4, shared weight) + exp (2 pair tiles)."""
                gi, b, j0, it0 = quad_info(t)
                for p in (0, 1):
                    col0 = (it0 + 2 * p) * TQ
                    if col0 % QCH == 0:
                        qin = qin_pool.tile([D, QCH], bf16, name="qin")
                        nc.sync.dma_start(out=qin,
                                          in_=qT[b, :, col0: col0 + QCH])
                        qin_tiles[b, col0 // QCH] = qin
                    qin = qin_tiles[b, col0 // QCH]
                    qo = col0 % QCH
                    stp = stp_pool.tile([KBA, 2 * TQ], f32, name="stp")
                    nc.tensor.matmul(stp[:, 0:TQ], kblk_sb[:, b, :],
                                     qin[:, qo: qo + TQ], start=True, stop=True)
                    nc.tensor.matmul(stp[:, TQ: 2 * TQ], kblk_sb[:, b, :],
                                     qin[:, qo + TQ: qo + 2 * TQ],
                                     start=True, stop=True)
                    stp_tiles[t, p] = stp
                for p in (0, 1):
                    etp = et_pool.tile([KBA, 2 * TQ], bf16, name="etp")
                    nc.scalar.activation(etp, stp_tiles.pop((t, p)),
                                         mybir.ActivationFunctionType.Exp,
                                         scale=1.0)
                    etp_tiles[t, p] = etp

            def stage_r4(t):
                """4 accumulating r4 matmuls (shifted selectors) for quad t."""
                gi, b, j0, _ = quad_info(t)
                if j0 == 0:
                    r4_tiles[gi] = r4_pool.tile([NPK, TQ], f32, name="r4b")
                r4b = r4_tiles[gi]
                for lj in range(4):
                    jg = j0 + lj               # group-local iteration 0..15
                    c0 = (GRP - 1 - jg) * NH5  # 75 - 5j
                    et = etp_tiles[t, lj // 2]
                    nc.tensor.matmul(
                        r4b, onesb_sb[:, b, c0: c0 + NPK],
                        et[:, (lj % 2) * TQ: (lj % 2 + 1) * TQ],
                        start=(jg == 0), stop=(jg == GRP - 1),
                        skip_group_check=True)

            def stage_recip(gi):
                """Group reciprocal + bf16 cast + HBM bounce + broadcast DMAs."""
                r4b = r4_tiles.pop(gi)
                rinvf = rinvf_pool.tile([NPK, TQ], f32, name="rinvf")
                nc.vector.reciprocal_approx_fast(rinvf, r4b)
                rinvb = rinvb_pool.tile([NPK, TQ], bf16, name="rinvb")
                nc.vector.tensor_copy(rinvb, rinvf)
                # bounce to HBM, reshuffled (j h) -> (h j) so each head's 16
                # iterations form one contiguous 16KB run
                nc.sync.dma_start(out=rb[gi].rearrange("h j q -> j h q"),
                                  in_=rinvb)
                rxb = rx_pool.tile([KBA, GRP, TQ], bf16, name="rxb")
                for h in range(H):
                    run = rb[gi, h].rearrange("j q -> (j q)")
                    nc.sync.dma_start(
                        out=rxb[NK * h: NK * (h + 1)].rearrange(
                            "p j q -> p (j q)"),
                        in_=run.unsqueeze(0).broadcast_to([NK, GRP * TQ]))
                nc.sync.dma_start(
                    out=rxb[KB: KBA].rearrange("p j q -> p (j q)"),
                    in_=ones8k.unsqueeze(0).broadcast_to([1, GRP * TQ]))
                rxb_tiles[gi] = rxb

            def stage_mul(t):
                """en = et * rx, one [125, 1024] bf16 multiply per pair."""
                gi, b, j0, _ = quad_info(t)
                rxb = rxb_tiles[gi]
                for p in (0, 1):
                    jg = j0 + 2 * p
                    enp = en_pool.tile([KBA, 2 * TQ], bf16, name="enp")
                    nc.vector.tensor_mul(
                        enp, etp_tiles.pop((t, p)),
                        rxb[:, jg: jg + 2, :].rearrange("k j q -> k (j q)"))
                    enp_tiles[t, p] = enp

            def stage_u(t):
                """4 u matmuls (shared weight, pair-packed) + copies + out DMA."""
                gi, b, j0, it0 = quad_info(t)
                us = []
                for p in (0, 1):
                    u = u_pool.tile([128, TQ], f32, name="u")
                    enp = enp_tiles.pop((t, p))
                    nc.tensor.matmul(u[0:D, :], vw_sb[:, b, :], enp[:, 0:TQ],
                                     start=True, stop=True)
                    nc.tensor.matmul(u[D: 2 * D, :], vw_sb[:, b, :],
                                     enp[:, TQ: 2 * TQ], start=True, stop=True)
                    us.append(u)
                for p in (0, 1):
                    it = it0 + 2 * p          # first iter of the pair
                    ot = it // 8              # output tile within batch
                    if it % 8 == 0:
                        osb_tiles[b, ot] = o_pool.tile([128, OCH], bf16,
                                                       name="osb")
                    osb = osb_tiles[b, ot]
                    po = (it % 8) // 2 * TQ
                    if copy_flip[0] % 3 != 2:
                        nc.vector.tensor_copy(osb[:, po: po + TQ], us[p])
                    else:
                        nc.scalar.copy(osb[:, po: po + TQ], us[p])
                    copy_flip[0] += 1
                    if it % 8 == 6:
                        nc.sync.dma_start(out=opk[b, ot], in_=osb)

            for t in range(nquads + LAG + 2):
                if t < nquads:
                    stage_a(t)
                if 0 <= t - 1 < nquads:
                    stage_r4(t - 1)
                    if (t - 1) % 4 == 3:
                        stage_recip((t - 1) // 4)
                if 0 <= t - LAG < nquads:
                    stage_mul(t - LAG)
                if 0 <= t - LAG - 1 < nquads:
                    stage_u(t - LAG - 1)

    nc.compile()
    return nc


def _get_program():
    if "v3" not in _PROG_CACHE:
        _PROG_CACHE["v3"] = _build_v3()
    return _PROG_CACHE["v3"]


def _host_prep(Q, K, V, mask, W_out, b_out):
    import ml_dtypes

    bf = ml_dtypes.bfloat16
    Q = np.asarray(Q, dtype=np.float32)
    K = np.asarray(K, dtype=np.float32)
    V = np.asarray(V, dtype=np.float32)
    W_out = np.asarray(W_out, dtype=np.float32)
    b_out = np.asarray(b_out, dtype=np.float32)
    mask = np.asarray(mask)
    m01 = mask.astype(np.float32)                     # [B, NK]

    Kblk = np.zeros((B, D, KBA), np.float32)
    VW = np.zeros((B, KBA, D), np.float32)
    onesb = np.zeros((B, KBA, OSEL), np.float32)
    P0 = (GRP - 1) * NH5                  # 75: selector block columns
    for h in range(H):
        ds, ks = h * DH, h * NK
        Kblk[:, ds: ds + DH, ks: ks + NK] = (
            K[:, :, ds: ds + DH].transpose(0, 2, 1) / SCALE
        )
        VW[:, ks: ks + NK, :] = (
            (V[:, :, ds: ds + DH] * m01[:, :, None]) @ W_out[:, ds: ds + DH].T
        )
        onesb[:, ks: ks + NK, P0 + h] = m01
    VW[:, KB, :] = b_out[None, :]

    QT = np.ascontiguousarray(Q.transpose(0, 2, 1)).astype(bf)   # [B, D, NQ]

    in_maps = []
    for c in range(NCORES):
        sl = slice(c * BL, (c + 1) * BL)
        in_maps.append(
            {
                "qT": QT[sl],
                "kblk": Kblk[sl].astype(bf),
                "vw": VW[sl].astype(bf),
                "onesb": onesb[sl].astype(bf),
                "ones8k": np.ones(GRP * TQ, np.float32).astype(bf),
            }
        )
    return in_maps


def _decode_out(res):
    out = np.empty((B, NQ, D), np.float32)
    for c in range(NCORES):
        o = np.asarray(res.results[c]["opk"], dtype=np.float32)
        # o: [BL, 4, 128, 2048] -> [BL, t, half, d, pair, qc]
        o = o.reshape(BL, NQ // (2 * OCH), 2, D, OCH // TQ, TQ)
        # q = ((t*pairs + pair)*2 + half)*TQ + qc
        o = o.transpose(0, 1, 4, 2, 5, 3)     # [BL, t, pair, half, qc, d]
        out[c * BL:(c + 1) * BL] = o.reshape(BL, NQ, D)
    return out


def _run(in_maps, **kwargs):
    nc = _get_program()
    return run_bass_kernel_spmd(nc, in_maps, list(range(NCORES)), **kwargs)


def kernel(Q, K, V, mask, W_out, b_out):
    in_maps = _host_prep(Q, K, V, mask, W_out, b_out)
    res = _run(in_maps)
    return _decode_out(res)


# revision 18
# speedup vs baseline: 2.7050x; 2.7050x over previous
"""Multi-head cross attention (B=32, Nq=16384, Nk=31, d_model=64, H=4) on 8 trn2 cores.

Data parallel over batch (4 per core). Per batch the attention is restructured so
the only large tensor (Q) is streamed once, host-pretransposed to qT [64, Nq] bf16:

  st  = Kblk_aug^T @ qt          Kblk_aug [64,125]: block-diag (K_h/sqrt(dh))^T,
                                 col 124 = 0 (so exp gives a constant ones row)
  et  = exp(st)                  no bias: the 0/1 mask is folded multiplicatively
                                 into onesb/VW below
  r4  = onesb_j^T @ et_j         16 iterations accumulate into one [80,512] PSUM
                                 bank via column-shifted selectors; rows 5j+h =
                                 masked head sums, rows 5j+4 unused
  rinv = 1/r4                    one reciprocal_approx_fast per 16 iterations
  rx  = PT_j^T @ rinvb           per-iteration broadcast matmul: PT_j [80,125]
                                 selects row 5j+h(k); col 124 uses a constant
                                 1.0 row appended to rinvb
  en  = et * rx                  softmax weights (+ ones row for the bias)
  u   = VW_aug^T @ en            VW_aug [125,64]: masked V_h @ W_out_h^T blocks,
                                 row 124 = b_out  -> u = out + bias
  out copies pack 2 iters into one [128,512] PSUM bank -> one copy per pair.

The loop is software-pipelined over quads (4 iterations): st/exp lead, r4 one
quad behind, rx/mul LAG quads behind, u one more behind — so every matmul's
inputs are long since ready and same-weight matmuls run in adjacent bursts
(a stationary-weight switch costs ~120ns of LDWEIGHTS serialization).
"""

import os
import sys

for _p in ("/opt/trn_rl_repo", "/opt/pypackages",
           "/root/.axon_site/_ro/trn_rl_repo", "/root/.axon_site/_ro/pypackages"):
    if os.path.isdir(_p) and _p not in sys.path:
        sys.path.insert(0, _p)

import math
import numpy as np

import concourse.bass as bass
import concourse.tile as tile
from concourse import bacc, mybir
from concourse.bass_utils import run_bass_kernel_spmd

B, NQ, NK, D = 32, 16384, 31, 64
H, DH = 4, 16
SCALE = math.sqrt(DH)
NCORES = 8
BL = B // NCORES          # batches per core
TQ = 512                  # queries per iteration
NT = NQ // TQ             # iterations per batch (32)
GRP = 16                  # iterations per reciprocal group
NG = NT // GRP            # groups per batch (2)
KB = H * NK               # 124 stacked key rows
KBA = KB + 1              # +1 ones/bias row
NH5 = H + 1               # 5 packed rows per iteration (4 heads + 1 pad)
NPK = GRP * NH5           # 80 packed r4 rows
NPK1 = NPK + 1            # +1 constant 1.0 row for the rx matmul
OSEL = 2 * GRP * NH5      # 160 columns of the padded selector

QCH = 4096                # q columns per input DMA (8 iters)
OCH = 2048                # q columns per output DMA tile (8 iters, row-packed)
LAG = 6                   # quads of pipeline lag between st/exp and rx/mul

_PROG_CACHE: dict = {}


def _build_v5():
    f32 = mybir.dt.float32
    bf16 = mybir.dt.bfloat16

    nc = bacc.Bacc("TRN2", target_bir_lowering=False, debug=False, num_devices=NCORES)
    qT = nc.dram_tensor("qT", [BL, D, NQ], bf16, kind="ExternalInput").ap()
    kblk = nc.dram_tensor("kblk", [BL, D, KBA], bf16, kind="ExternalInput").ap()
    vw = nc.dram_tensor("vw", [BL, KBA, D], bf16, kind="ExternalInput").ap()
    onesb = nc.dram_tensor("onesb", [BL, KBA, OSEL], bf16,
                           kind="ExternalInput").ap()
    ptall = nc.dram_tensor("ptall", [NPK1, GRP, KBA], bf16,
                           kind="ExternalInput").ap()
    onestq = nc.dram_tensor("onestq", [1, TQ], bf16, kind="ExternalInput").ap()
    n_otile = NQ * D // (128 * OCH)     # 4 output tiles per batch
    opk = nc.dram_tensor("opk", [BL, n_otile, 128, OCH], bf16,
                         kind="ExternalOutput").ap()

    nquads = BL * NG * (GRP // 4)       # 32 quad steps

    with tile.TileContext(nc) as tc:
        with (
            tc.tile_pool(name="singles", bufs=1) as singles,
            tc.tile_pool(name="qin", bufs=3) as qin_pool,
            tc.tile_pool(name="stp", bufs=3, space="PSUM") as stp_pool,
            tc.tile_pool(name="etp", bufs=4 * LAG + 8) as et_pool,
            tc.tile_pool(name="r4", bufs=1, space="PSUM") as r4_pool,
            tc.tile_pool(name="rinvf", bufs=2) as rinvf_pool,
            tc.tile_pool(name="rinvb", bufs=2) as rinvb_pool,
            tc.tile_pool(name="rx", bufs=2, space="PSUM") as rx_pool,
            tc.tile_pool(name="enp", bufs=8) as en_pool,
            tc.tile_pool(name="u", bufs=2, space="PSUM") as u_pool,
            tc.tile_pool(name="osb", bufs=3) as o_pool,
        ):
            kblk_sb = singles.tile([D, BL, KBA], bf16)
            vw_sb = singles.tile([KBA, BL, D], bf16)
            onesb_sb = singles.tile([KBA, BL, OSEL], bf16)
            pt_sb = singles.tile([NPK1, GRP, KBA], bf16)
            for b in range(BL):
                nc.sync.dma_start(out=kblk_sb[:, b, :], in_=kblk[b])
                nc.sync.dma_start(out=vw_sb[:, b, :], in_=vw[b])
                nc.sync.dma_start(out=onesb_sb[:, b, :], in_=onesb[b])
            nc.sync.dma_start(out=pt_sb, in_=ptall)

            qin_tiles = {}
            st_tiles = {}
            et_tiles = {}
            r4_tiles = {}
            rinvb_tiles = {}
            en_tiles = {}
            u_tiles = {}
            osb_tiles = {}
            copy_flip = [0]

            def quad_info(t):
                gi = t // 4                # global group index
                b = gi // NG               # batch on this core
                j0 = (t % 4) * 4           # first group-local iteration
                it0 = (gi % NG) * GRP + j0  # first batch-local iteration
                return gi, b, j0, it0

            def stage_a(t, half):
                """st matmuls + exp; half 0 = iterations 0..2, half 1 = 3."""
                gi, b, j0, it0 = quad_info(t)
                rng = range(3) if half == 0 else range(3, 4)
                for lj in rng:
                    it = it0 + lj
                    col0 = it * TQ
                    if col0 % QCH == 0:
                        qin = qin_pool.tile([D, QCH], bf16, name="qin")
                        nc.sync.dma_start(out=qin,
                                          in_=qT[b, :, col0: col0 + QCH])
                        qin_tiles[b, col0 // QCH] = qin
                    qin = qin_tiles[b, col0 // QCH]
                    qo = col0 % QCH
                    st = stp_pool.tile([KBA, TQ], f32, name="st")
                    nc.tensor.matmul(st, kblk_sb[:, b, :],
                                     qin[:, qo: qo + TQ], start=True, stop=True)
                    et = et_pool.tile([KBA, TQ], bf16, name="et")
                    nc.scalar.activation(et, st,
                                         mybir.ActivationFunctionType.Exp,
                                         scale=1.0)
                    et_tiles[t, lj] = et

            def stage_r4(t):
                """4 accumulating r4 matmuls (shifted selectors) for quad t."""
                gi, b, j0, _ = quad_info(t)
                if j0 == 0:
                    r4_tiles[gi] = r4_pool.tile([NPK, TQ], f32, name="r4b")
                r4b = r4_tiles[gi]
                for lj in range(4):
                    jg = j0 + lj               # group-local iteration 0..15
                    c0 = (GRP - 1 - jg) * NH5  # 75 - 5j
                    nc.tensor.matmul(
                        r4b, onesb_sb[:, b, c0: c0 + NPK], et_tiles[t, lj],
                        start=(jg == 0), stop=(jg == GRP - 1),
                        skip_group_check=True)

            def stage_recip(gi):
                """Group reciprocal + bf16 cast (+ constant 1.0 row 80)."""
                r4b = r4_tiles.pop(gi)
                rinvf = rinvf_pool.tile([NPK, TQ], f32, name="rinvf")
                nc.vector.reciprocal_approx_fast(rinvf, r4b)
                rinvb = rinvb_pool.tile([NPK1, TQ], bf16, name="rinvb")
                nc.vector.tensor_copy(rinvb[0:NPK, :], rinvf)
                nc.sync.dma_start(out=rinvb[NPK: NPK1, :], in_=onestq)
                rinvb_tiles[gi] = rinvb

            def stage_rx_mul(t):
                """Per-iteration broadcast matmul + normalization multiply."""
                gi, b, j0, _ = quad_info(t)
                rinvb = rinvb_tiles[gi]
                for lj in range(4):
                    jg = j0 + lj
                    rx = rx_pool.tile([KBA, TQ], f32, name="rx")
                    nc.tensor.matmul(rx, pt_sb[:, jg, :], rinvb,
                                     start=True, stop=True)
                    en = en_pool.tile([KBA, TQ], bf16, name="en")
                    nc.vector.tensor_mul(en, et_tiles.pop((t, lj)), rx)
                    en_tiles[t, lj] = en

            def stage_u(t):
                """4 u matmuls (shared weight, pair-packed) + copies + out DMA."""
                gi, b, j0, it0 = quad_info(t)
                us = []
                for p in (0, 1):
                    u = u_pool.tile([128, TQ], f32, name="u")
                    nc.tensor.matmul(u[0:D, :], vw_sb[:, b, :],
                                     en_tiles.pop((t, 2 * p)),
                                     start=True, stop=True)
                    nc.tensor.matmul(u[D: 2 * D, :], vw_sb[:, b, :],
                                     en_tiles.pop((t, 2 * p + 1)),
                                     start=True, stop=True)
                    us.append(u)
                for p in (0, 1):
                    it = it0 + 2 * p          # first iter of the pair
                    ot = it // 8              # output tile within batch
                    if it % 8 == 0:
                        osb_tiles[b, ot] = o_pool.tile([128, OCH], bf16,
                                                       name="osb")
                    osb = osb_tiles[b, ot]
                    po = (it % 8) // 2 * TQ
                    if copy_flip[0] % 3 != 2:
                        nc.vector.tensor_copy(osb[:, po: po + TQ], us[p])
                    else:
                        nc.scalar.copy(osb[:, po: po + TQ], us[p])
                    copy_flip[0] += 1
                    if it % 8 == 6:
                        nc.sync.dma_start(out=opk[b, ot], in_=osb)

            for t in range(nquads + LAG + 2):
                if t < nquads:
                    stage_a(t, 0)
                if 0 <= t - 1 < nquads:
                    stage_r4(t - 1)
                if t < nquads:
                    stage_a(t, 1)
                if 0 <= t - 1 < nquads and (t - 1) % 4 == 3:
                    stage_recip((t - 1) // 4)
                if 0 <= t - LAG < nquads:
                    stage_rx_mul(t - LAG)
                if 0 <= t - LAG - 1 < nquads:
                    stage_u(t - LAG - 1)

    nc.compile()
    return nc


def _get_program():
    if "v5" not in _PROG_CACHE:
        _PROG_CACHE["v5"] = _build_v5()
    return _PROG_CACHE["v5"]


def _host_prep(Q, K, V, mask, W_out, b_out):
    import ml_dtypes

    bf = ml_dtypes.bfloat16
    Q = np.asarray(Q, dtype=np.float32)
    K = np.asarray(K, dtype=np.float32)
    V = np.asarray(V, dtype=np.float32)
    W_out = np.asarray(W_out, dtype=np.float32)
    b_out = np.asarray(b_out, dtype=np.float32)
    mask = np.asarray(mask)
    m01 = mask.astype(np.float32)                     # [B, NK]

    Kblk = np.zeros((B, D, KBA), np.float32)
    VW = np.zeros((B, KBA, D), np.float32)
    onesb = np.zeros((B, KBA, OSEL), np.float32)
    PTall = np.zeros((NPK1, GRP, KBA), np.float32)
    P0 = (GRP - 1) * NH5                  # 75: selector block columns
    for h in range(H):
        ds, ks = h * DH, h * NK
        Kblk[:, ds: ds + DH, ks: ks + NK] = (
            K[:, :, ds: ds + DH].transpose(0, 2, 1) / SCALE
        )
        VW[:, ks: ks + NK, :] = (
            (V[:, :, ds: ds + DH] * m01[:, :, None]) @ W_out[:, ds: ds + DH].T
        )
        onesb[:, ks: ks + NK, P0 + h] = m01
        for j in range(GRP):
            PTall[j * NH5 + h, j, ks: ks + NK] = 1.0
    VW[:, KB, :] = b_out[None, :]
    # bias pseudo-head: r4 rows 5j+4 = et[124] = 1.0 so no packed row is ever
    # 0 (reciprocal(0) is undefined and would NaN-poison the rx matmul)
    onesb[:, KB, P0 + H] = 1.0
    PTall[NPK, :, KB] = 1.0               # rx row 124 <- constant 1.0 row

    QT = np.ascontiguousarray(Q.transpose(0, 2, 1)).astype(bf)   # [B, D, NQ]

    in_maps = []
    for c in range(NCORES):
        sl = slice(c * BL, (c + 1) * BL)
        in_maps.append(
            {
                "qT": QT[sl],
                "kblk": Kblk[sl].astype(bf),
                "vw": VW[sl].astype(bf),
                "onesb": onesb[sl].astype(bf),
                "ptall": PTall.astype(bf),
                "onestq": np.ones((1, TQ), np.float32).astype(bf),
            }
        )
    return in_maps


def _decode_out(res):
    out = np.empty((B, NQ, D), np.float32)
    for c in range(NCORES):
        o = np.asarray(res.results[c]["opk"], dtype=np.float32)
        # o: [BL, 4, 128, 2048] -> [BL, t, half, d, pair, qc]
        o = o.reshape(BL, NQ // (2 * OCH), 2, D, OCH // TQ, TQ)
        # q = ((t*pairs + pair)*2 + half)*TQ + qc
        o = o.transpose(0, 1, 4, 2, 5, 3)     # [BL, t, pair, half, qc, d]
        out[c * BL:(c + 1) * BL] = o.reshape(BL, NQ, D)
    return out


def _run(in_maps, **kwargs):
    nc = _get_program()
    return run_bass_kernel_spmd(nc, in_maps, list(range(NCORES)), **kwargs)


def kernel(Q, K, V, mask, W_out, b_out):
    in_maps = _host_prep(Q, K, V, mask, W_out, b_out)
    res = _run(in_maps)
    return _decode_out(res)


# revision 19
# speedup vs baseline: 3.1355x; 1.1592x over previous
"""Multi-head cross attention (B=32, Nq=16384, Nk=31, d_model=64, H=4) on 8 trn2 cores.

Data parallel over batch (4 per core). Per batch the attention is restructured so
the only large tensor (Q) is streamed once, host-pretransposed to qT [64, Nq] bf16:

  st  = Kblk_aug^T @ qt          Kblk_aug [64,125]: block-diag (K_h/sqrt(dh))^T,
                                 col 124 = 0 (so exp gives a constant ones row)
  et  = exp(st)                  no bias: the 0/1 mask is folded multiplicatively
                                 into onesb/VW below
  r4  = onesb_j^T @ et_j         16 iterations accumulate into one [80,512] PSUM
                                 bank via column-shifted selectors; rows 5j+h =
                                 masked head sums, rows 5j+4 unused
  rinv = 1/r4                    one reciprocal_approx_fast per 16 iterations
  rx  = PT_j^T @ rinvb           per-iteration broadcast matmul: PT_j [80,125]
                                 selects row 5j+h(k); col 124 uses a constant
                                 1.0 row appended to rinvb
  en  = et * rx                  softmax weights (+ ones row for the bias)
  u   = VW_aug^T @ en            VW_aug [125,64]: masked V_h @ W_out_h^T blocks,
                                 row 124 = b_out  -> u = out + bias
  out copies pack 2 iters into one [128,512] PSUM bank -> one copy per pair.

The loop is software-pipelined over quads (4 iterations): st/exp lead, r4 one
quad behind, rx/mul LAG quads behind, u one more behind — so every matmul's
inputs are long since ready and same-weight matmuls run in adjacent bursts
(a stationary-weight switch costs ~120ns of LDWEIGHTS serialization).
"""

import os
import sys

for _p in ("/opt/trn_rl_repo", "/opt/pypackages",
           "/root/.axon_site/_ro/trn_rl_repo", "/root/.axon_site/_ro/pypackages"):
    if os.path.isdir(_p) and _p not in sys.path:
        sys.path.insert(0, _p)

import math
import numpy as np

import concourse.bass as bass
import concourse.tile as tile
from concourse import bacc, mybir
from concourse.bass_utils import run_bass_kernel_spmd

B, NQ, NK, D = 32, 16384, 31, 64
H, DH = 4, 16
SCALE = math.sqrt(DH)
NCORES = 8
BL = B // NCORES          # batches per core
TQ = 512                  # queries per iteration
NT = NQ // TQ             # iterations per batch (32)
GRP = 16                  # iterations per reciprocal group
NG = NT // GRP            # groups per batch (2)
KB = H * NK               # 124 stacked key rows
KBA = KB + 1              # +1 ones/bias row
NH5 = H + 1               # 5 packed rows per iteration (4 heads + 1 pad)
NPK = GRP * NH5           # 80 packed r4 rows
NPK1 = NPK + 1            # +1 constant 1.0 row for the rx matmul
OSEL = 2 * GRP * NH5      # 160 columns of the padded selector

QCH = 4096                # q columns per input DMA (8 iters)
OCH = 2048                # q columns per output DMA tile (8 iters, row-packed)
LAG = 6                   # quads of pipeline lag between st/exp and rx/mul

_PROG_CACHE: dict = {}


def _build_v5():
    f32 = mybir.dt.float32
    bf16 = mybir.dt.bfloat16

    nc = bacc.Bacc("TRN2", target_bir_lowering=False, debug=False, num_devices=NCORES)
    qT = nc.dram_tensor("qT", [BL, D, NQ], bf16, kind="ExternalInput").ap()
    kblk = nc.dram_tensor("kblk", [BL, D, KBA], bf16, kind="ExternalInput").ap()
    vw = nc.dram_tensor("vw", [BL, KBA, D], bf16, kind="ExternalInput").ap()
    onesb = nc.dram_tensor("onesb", [BL, KBA, OSEL], bf16,
                           kind="ExternalInput").ap()
    ptall = nc.dram_tensor("ptall", [NPK1, GRP, KBA], bf16,
                           kind="ExternalInput").ap()
    onestq = nc.dram_tensor("onestq", [1, TQ], bf16, kind="ExternalInput").ap()
    n_otile = NQ * D // (128 * OCH)     # 4 output tiles per batch
    opk = nc.dram_tensor("opk", [BL, n_otile, 128, OCH], bf16,
                         kind="ExternalOutput").ap()

    nquads = BL * NG * (GRP // 4)       # 32 quad steps

    with tile.TileContext(nc) as tc:
        with (
            tc.tile_pool(name="singles", bufs=1) as singles,
            tc.tile_pool(name="qin", bufs=3) as qin_pool,
            tc.tile_pool(name="stp", bufs=3, space="PSUM") as stp_pool,
            tc.tile_pool(name="etp", bufs=4 * LAG + 8) as et_pool,
            tc.tile_pool(name="r4", bufs=1, space="PSUM") as r4_pool,
            tc.tile_pool(name="rinvf", bufs=2) as rinvf_pool,
            tc.tile_pool(name="rinvb", bufs=2) as rinvb_pool,
            tc.tile_pool(name="rx", bufs=2, space="PSUM") as rx_pool,
            tc.tile_pool(name="enp", bufs=8) as en_pool,
            tc.tile_pool(name="u", bufs=2, space="PSUM") as u_pool,
            tc.tile_pool(name="osb", bufs=3) as o_pool,
        ):
            kblk_sb = singles.tile([D, BL, KBA], bf16)
            vw_sb = singles.tile([KBA, BL, D], bf16)
            onesb_sb = singles.tile([KBA, BL, OSEL], bf16)
            pt_sb = singles.tile([NPK1, GRP, KBA], bf16)
            for b in range(BL):
                nc.sync.dma_start(out=kblk_sb[:, b, :], in_=kblk[b])
                nc.sync.dma_start(out=vw_sb[:, b, :], in_=vw[b])
                nc.sync.dma_start(out=onesb_sb[:, b, :], in_=onesb[b])
            nc.sync.dma_start(out=pt_sb, in_=ptall)

            qin_tiles = {}
            st_tiles = {}
            et_tiles = {}
            r4_tiles = {}
            rinvb_tiles = {}
            en_tiles = {}
            u_tiles = {}
            osb_tiles = {}
            copy_flip = [0]

            def quad_info(t):
                gi = t // 4                # global group index
                b = gi // NG               # batch on this core
                j0 = (t % 4) * 4           # first group-local iteration
                it0 = (gi % NG) * GRP + j0  # first batch-local iteration
                return gi, b, j0, it0

            def stage_a(t, half):
                """st matmuls + exp; half 0 = iterations 0..2, half 1 = 3."""
                gi, b, j0, it0 = quad_info(t)
                rng = range(3) if half == 0 else range(3, 4)
                for lj in rng:
                    it = it0 + lj
                    col0 = it * TQ
                    if col0 % QCH == 0:
                        qin = qin_pool.tile([D, QCH], bf16, name="qin")
                        nc.sync.dma_start(out=qin,
                                          in_=qT[b, :, col0: col0 + QCH])
                        qin_tiles[b, col0 // QCH] = qin
                    qin = qin_tiles[b, col0 // QCH]
                    qo = col0 % QCH
                    st = stp_pool.tile([KBA, TQ], f32, name="st")
                    nc.tensor.matmul(st, kblk_sb[:, b, :],
                                     qin[:, qo: qo + TQ], start=True, stop=True)
                    et = et_pool.tile([KBA, TQ], bf16, name="et")
                    nc.scalar.activation(et, st,
                                         mybir.ActivationFunctionType.Exp,
                                         scale=1.0)
                    et_tiles[t, lj] = et

            def stage_r4(t):
                """4 accumulating r4 matmuls (shifted selectors) for quad t."""
                gi, b, j0, _ = quad_info(t)
                if j0 == 0:
                    r4_tiles[gi] = r4_pool.tile([NPK, TQ], f32, name="r4b")
                r4b = r4_tiles[gi]
                for lj in range(4):
                    jg = j0 + lj               # group-local iteration 0..15
                    c0 = (GRP - 1 - jg) * NH5  # 75 - 5j
                    nc.tensor.matmul(
                        r4b, onesb_sb[:, b, c0: c0 + NPK], et_tiles[t, lj],
                        start=(jg == 0), stop=(jg == GRP - 1),
                        skip_group_check=True)

            def stage_recip(gi):
                """Group reciprocal + bf16 cast (+ constant 1.0 row 80)."""
                r4b = r4_tiles.pop(gi)
                rinvf = rinvf_pool.tile([NPK, TQ], f32, name="rinvf")
                nc.vector.reciprocal_approx_fast(rinvf, r4b)
                rinvb = rinvb_pool.tile([NPK1, TQ], bf16, name="rinvb")
                nc.vector.tensor_copy(rinvb[0:NPK, :], rinvf)
                nc.sync.dma_start(out=rinvb[NPK: NPK1, :], in_=onestq)
                rinvb_tiles[gi] = rinvb

            def stage_rx_mul(t):
                """Per-iteration broadcast matmul + normalization multiply."""
                gi, b, j0, _ = quad_info(t)
                rinvb = rinvb_tiles[gi]
                for lj in range(4):
                    jg = j0 + lj
                    rx = rx_pool.tile([KBA, TQ], f32, name="rx")
                    nc.tensor.matmul(rx, pt_sb[:, jg, :], rinvb,
                                     start=True, stop=True)
                    en = en_pool.tile([KBA, TQ], bf16, name="en")
                    nc.vector.tensor_mul(en, et_tiles.pop((t, lj)), rx)
                    en_tiles[t, lj] = en

            def stage_u(t):
                """4 u matmuls (shared weight, pair-packed) + copies + out DMA."""
                gi, b, j0, it0 = quad_info(t)
                us = []
                for p in (0, 1):
                    u = u_pool.tile([128, TQ], f32, name="u")
                    nc.tensor.matmul(u[0:D, :], vw_sb[:, b, :],
                                     en_tiles.pop((t, 2 * p)),
                                     start=True, stop=True)
                    nc.tensor.matmul(u[D: 2 * D, :], vw_sb[:, b, :],
                                     en_tiles.pop((t, 2 * p + 1)),
                                     start=True, stop=True)
                    us.append(u)
                for p in (0, 1):
                    it = it0 + 2 * p          # first iter of the pair
                    ot = it // 8              # output tile within batch
                    if it % 8 == 0:
                        osb_tiles[b, ot] = o_pool.tile([128, OCH], bf16,
                                                       name="osb")
                    osb = osb_tiles[b, ot]
                    po = (it % 8) // 2 * TQ
                    if copy_flip[0] % 3 != 2:
                        nc.vector.tensor_copy(osb[:, po: po + TQ], us[p])
                    else:
                        nc.scalar.copy(osb[:, po: po + TQ], us[p])
                    copy_flip[0] += 1
                    if it % 8 == 6:
                        nc.sync.dma_start(out=opk[b, ot], in_=osb)

            # Within a step, emit PE work whose dependencies are oldest first
            # (u lags LAG+1 quads, rx LAG quads) so the PE queue always has
            # runnable matmuls while the current step's st/exp/r4 data flows.
            for t in range(nquads + LAG + 2):
                if 0 <= t - LAG - 1 < nquads:
                    stage_u(t - LAG - 1)
                if 0 <= t - LAG < nquads:
                    stage_rx_mul(t - LAG)
                if t < nquads:
                    stage_a(t, 0)
                if 0 <= t - 1 < nquads:
                    stage_r4(t - 1)
                if t < nquads:
                    stage_a(t, 1)
                if 0 <= t - 1 < nquads and (t - 1) % 4 == 3:
                    stage_recip((t - 1) // 4)

    nc.compile()
    return nc


def _get_program():
    if "v5" not in _PROG_CACHE:
        _PROG_CACHE["v5"] = _build_v5()
    return _PROG_CACHE["v5"]


def _host_prep(Q, K, V, mask, W_out, b_out):
    import ml_dtypes

    bf = ml_dtypes.bfloat16
    Q = np.asarray(Q, dtype=np.float32)
    K = np.asarray(K, dtype=np.float32)
    V = np.asarray(V, dtype=np.float32)
    W_out = np.asarray(W_out, dtype=np.float32)
    b_out = np.asarray(b_out, dtype=np.float32)
    mask = np.asarray(mask)
    m01 = mask.astype(np.float32)                     # [B, NK]

    Kblk = np.zeros((B, D, KBA), np.float32)
    VW = np.zeros((B, KBA, D), np.float32)
    onesb = np.zeros((B, KBA, OSEL), np.float32)
    PTall = np.zeros((NPK1, GRP, KBA), np.float32)
    P0 = (GRP - 1) * NH5                  # 75: selector block columns
    for h in range(H):
        ds, ks = h * DH, h * NK
        Kblk[:, ds: ds + DH, ks: ks + NK] = (
            K[:, :, ds: ds + DH].transpose(0, 2, 1) / SCALE
        )
        VW[:, ks: ks + NK, :] = (
            (V[:, :, ds: ds + DH] * m01[:, :, None]) @ W_out[:, ds: ds + DH].T
        )
        onesb[:, ks: ks + NK, P0 + h] = m01
        for j in range(GRP):
            PTall[j * NH5 + h, j, ks: ks + NK] = 1.0
    VW[:, KB, :] = b_out[None, :]
    # bias pseudo-head: r4 rows 5j+4 = et[124] = 1.0 so no packed row is ever
    # 0 (reciprocal(0) is undefined and would NaN-poison the rx matmul)
    onesb[:, KB, P0 + H] = 1.0
    PTall[NPK, :, KB] = 1.0               # rx row 124 <- constant 1.0 row

    QT = np.ascontiguousarray(Q.transpose(0, 2, 1)).astype(bf)   # [B, D, NQ]

    in_maps = []
    for c in range(NCORES):
        sl = slice(c * BL, (c + 1) * BL)
        in_maps.append(
            {
                "qT": QT[sl],
                "kblk": Kblk[sl].astype(bf),
                "vw": VW[sl].astype(bf),
                "onesb": onesb[sl].astype(bf),
                "ptall": PTall.astype(bf),
                "onestq": np.ones((1, TQ), np.float32).astype(bf),
            }
        )
    return in_maps


def _decode_out(res):
    out = np.empty((B, NQ, D), np.float32)
    for c in range(NCORES):
        o = np.asarray(res.results[c]["opk"], dtype=np.float32)
        # o: [BL, 4, 128, 2048] -> [BL, t, half, d, pair, qc]
        o = o.reshape(BL, NQ // (2 * OCH), 2, D, OCH // TQ, TQ)
        # q = ((t*pairs + pair)*2 + half)*TQ + qc
        o = o.transpose(0, 1, 4, 2, 5, 3)     # [BL, t, pair, half, qc, d]
        out[c * BL:(c + 1) * BL] = o.reshape(BL, NQ, D)
    return out


def _run(in_maps, **kwargs):
    nc = _get_program()
    return run_bass_kernel_spmd(nc, in_maps, list(range(NCORES)), **kwargs)


def kernel(Q, K, V, mask, W_out, b_out):
    in_maps = _host_prep(Q, K, V, mask, W_out, b_out)
    res = _run(in_maps)
    return _decode_out(res)
